# revision 19
# baseline (speedup 1.0000x reference)
"""Trainium2 Bass kernel for nn_ConvLSTMNet (bidirectional per-pixel ConvLSTM + FC stack).

Strategy (v2)
-------------
* Data-parallel over batch: 8 cores x 4 samples; each core runs both x1 and x2
  sub-forwards (shared weights) = 8 chains x 55 px = 440 recurrence columns
  (+1 zero pad col), all independent until the epilogue.
* FC stack 7040->3400->1000->500->50 has no nonlinearities -> collapsed on host
  into one 7040->50 matrix (f64 accumulate); tiny on-device GEMM at the end.
* Columns split into G=3 phase groups of 147 cols, pipelined so the Activation
  engine stream is sigma_g(t), tanh_{g-1}(t), sigma_{g+1}(t), ... with each
  instruction's input ready before the engine reaches it (Act is the
  throughput-limiting engine: cost = elems*0.83ns + ~185ns init per instr).
* Both cells merged per matmul via block-diagonal weights: per gate one
  K=128 h-matmul ([wh1 0; 0 wh2], rhs = [h1;h2]) and one K=6 x-matmul
  ([wx1|b1 0; 0 wx2|b2] with a ones-row carrying the bias; cell2's x data
  pre-reversed in time on the host) -> 8 matmuls per group-step instead of 16.
* PSUM: 2 gates per bank ([f|i] and [o|g]), 2 banks per group, 6 banks total.
  One sigmoid per group-step over all 4 gates (tanh(g) computed as
  2*sigmoid(2g)-1 with pre-doubled g weights), one Tanh(c), 4 DVE ops.
"""

import os
import sys

try:
    import concourse.bass  # noqa: F401  (provided by the environment boot)
except ImportError:  # fallback for bare environments
    sys.path.insert(0, "/opt/trn_rl_repo")

import numpy as np
import ml_dtypes

import concourse.bass as bass
import concourse.bacc as bacc
import concourse.tile as tile
from concourse import mybir
from concourse.bass_utils import run_bass_kernel_spmd

# ---------------------------------------------------------------- constants
B, T_FULL, IC, H, W = 32, 256, 2, 5, 11
P = H * W            # 55
HC = 64
N_CORES = 8
BL = B // N_CORES    # 4 samples per core
NCHAIN = 2 * BL      # 8 chains per core (x1/x2 x samples)
G = 3                # phase groups (column split)
NCOL = 147           # columns per group (3*147 = 441 = 440 + 1 pad)
NCT = G * NCOL       # 441
FC_OUT = 50

GATE_SL = {"i": (0, 64), "f": (64, 128), "o": (128, 192), "g": (192, 256)}
BANKS = ["f", "i", "o", "g"]  # per-group gate order; "g" is pre-doubled

F32 = mybir.dt.float32
BF16 = mybir.dt.bfloat16

GDT = BF16   # sigmoid/tanh outputs, h/R
CDT = BF16 if os.environ.get("K_CDT", "bf16") == "bf16" else F32
XDT = BF16   # staged x data
WDT = BF16   # recurrence weights

_NPDT = {F32: np.float32, BF16: ml_dtypes.bfloat16}


def _np(dt):
    return _NPDT[dt]


# ---------------------------------------------------------------- device build
_BUILD_CACHE = {}


def _build(t_steps: int):
    """Build + compile the per-core Bass module (cached)."""
    key = (t_steps,)
    if key in _BUILD_CACHE:
        return _BUILD_CACHE[key]

    n_tb = (t_steps + 63) // 64  # number of 64-step time blocks (4 for T=256)
    assert n_tb <= 4

    nc = bacc.Bacc("TRN2", target_bir_lowering=False, debug=False,
                   num_devices=N_CORES)

    # xg: per time-block j (partition rows 32j..32j+11): a [12, 2*NCOL]
    # block-diagonal x layout per (group, step-in-block): rows 0-5 hold
    # (x0, x1, ones, x0r, x1r, ones) for the even gate's columns [0:NCOL],
    # rows 6-11 the same data for the odd gate's columns [NCOL:2*NCOL]
    # (zeros elsewhere). One K=12 x-matmul then fills a whole psum bank
    # (both gates), so each bank gets exactly ONE start_tensor_calc per
    # step (start marks the whole 2KB zero region pending-zero) and the
    # x-matmuls can dispatch before R is ready.
    xg_d = nc.dram_tensor("xg", [n_tb, 12, G, 64, 2 * NCOL], XDT,
                          kind="ExternalInput").ap()
    wh_d = nc.dram_tensor("wh", [128, 512], WDT, kind="ExternalInput").ap()
    wx_d = nc.dram_tensor("wx", [128, 256], WDT, kind="ExternalInput").ap()
    weff_d = nc.dram_tensor("weff", [128, P * FC_OUT], WDT,
                            kind="ExternalInput").ap()
    beff_d = nc.dram_tensor("beff", [FC_OUT, 1], F32, kind="ExternalInput").ap()
    out_d = nc.dram_tensor("out", [2, BL, FC_OUT], F32,
                           kind="ExternalOutput").ap()

    from contextlib import ExitStack

    with tile.TileContext(nc) as tc, ExitStack() as top:
        # ---------------- persistent SBUF tiles
        singles = top.enter_context(tc.tile_pool(name="singles", bufs=1))
        xg_sb = singles.tile([128, G, 64, 2 * NCOL], XDT, name="xg_sb")
        wh_sb = singles.tile([128, 512], WDT, name="wh_sb")
        wx_sb = singles.tile([128, 256], WDT, name="wx_sb")
        weff_sb = singles.tile([128, P * FC_OUT], WDT, name="weff_sb")
        beff_sb = singles.tile([FC_OUT, 1], F32, name="beff_sb")

        Rg, cg = [], []
        for g in range(G):
            Rg.append(singles.tile([128, NCOL], GDT, name=f"R{g}"))
            cg.append(singles.tile([128, NCOL], CDT, name=f"c{g}"))

        # ---------------- loads + state init
        for j in range(n_tb):
            nc.sync.dma_start(out=xg_sb[32 * j:32 * j + 12],
                              in_=xg_d[j])
        nc.sync.dma_start(out=wh_sb[:], in_=wh_d[:])
        nc.sync.dma_start(out=wx_sb[:], in_=wx_d[:])
        nc.sync.dma_start(out=weff_sb[:], in_=weff_d[:])
        nc.sync.dma_start(out=beff_sb[:], in_=beff_d[:])
        for g in range(G):
            nc.vector.memset(Rg[g][:], 0.0)
            nc.vector.memset(cg[g][:], 0.0)

        # ---------------- pools for psum + per-step intermediates
        es = ExitStack()
        pspool = es.enter_context(
            tc.tile_pool(name="psmain", bufs=1, space="PSUM"))
        # per group: 2 banks, 2 gates per bank: [f|i], [o|g]
        ps = [pspool.tile([128, 1024], F32, name=f"ps{g}") for g in range(G)]
        pools = {}
        for g in range(G):
            for nm in ("sg", "vp", "u", "tct"):
                pools[(nm, g)] = es.enter_context(
                    tc.tile_pool(name=f"{nm}{g}", bufs=2))

        sg_cur = [None] * G   # current sigmoid-output tile per group

        def bank_ap(g: int, k: int):
            return ps[g][:, 512 * (k // 2) + NCOL * (k % 2):
                         512 * (k // 2) + NCOL * (k % 2) + NCOL]

        def emit_front(g: int, t: int):
            """mm(g,t), sigma(g,t), DVE vp/u/c (g,t).

            The 2 bank-wide x-matmuls are emitted before the h-matmuls:
            they depend only on sigma(g,t-1) having read the banks, so the
            in-order PE SEQ dispatches them early and only the 4 h-matmuls
            sit between R(g,t-1) landing and sigma(g,t)."""
            j, s = t // 64, t % 64
            r0 = 32 * j
            for b in range(2):
                nc.tensor.matmul(ps[g][:, 512 * b:512 * b + 2 * NCOL],
                                 wx_sb[r0:r0 + 12, 128 * b:128 * b + 128],
                                 xg_sb[r0:r0 + 12, g, s, :],
                                 start=True, stop=False,
                                 tile_position=(r0, 0))
            for k in range(4):
                nc.tensor.matmul(bank_ap(g, k),
                                 wh_sb[:, 128 * k:128 * k + 128],
                                 Rg[g][:],
                                 start=False, stop=(k % 2 == 1))

            sg = pools[("sg", g)].tile([128, 4, NCOL], GDT, name=f"sgt{g}")
            psv = ps[g].rearrange("p (b n) -> p b n", b=2)[:, :, 0:2 * NCOL]
            nc.scalar.activation(sg.rearrange("p f n -> p (f n)"), psv,
                                 mybir.ActivationFunctionType.Sigmoid)
            sg_cur[g] = sg

            # gate views: bank0 = [f|i], bank1 = [o|g]
            s_f, s_i = sg[:, 0, :], sg[:, 1, :]
            s_g = sg[:, 3, :]
            vp = pools[("vp", g)].tile([128, NCOL], GDT, name=f"vpt{g}")
            # vp = (sig(2g) - 0.5) * sig(i)   ( = tanh(g)*sig(i)/2 )
            nc.vector.scalar_tensor_tensor(vp[:], s_g, 0.5, s_i,
                                           mybir.AluOpType.subtract,
                                           mybir.AluOpType.mult)
            u = pools[("u", g)].tile([128, NCOL], CDT, name=f"ut{g}")
            nc.vector.tensor_mul(u[:], s_f, cg[g][:])     # sig(f)*c
            # c = 2*vp + u
            nc.vector.scalar_tensor_tensor(cg[g][:], vp[:], 2.0, u[:],
                                           mybir.AluOpType.mult,
                                           mybir.AluOpType.add)

        def emit_back(g: int):
            """tanh(c_g) then h(g) = sig(o)*tanh(c) -> R(g).

            Emitted one group-slot after front(g): the Act engine reaches this
            tanh only after the next group's sigmoid, by which time c_g is
            ready -- no in-order blocking on the Act queue.
            """
            tct = pools[("tct", g)].tile([128, NCOL], GDT, name=f"tctt{g}")
            nc.scalar.activation(tct[:], cg[g][:],
                                 mybir.ActivationFunctionType.Tanh)
            s_o = sg_cur[g][:, 2, :]
            nc.vector.tensor_mul(Rg[g][:], s_o, tct[:])

        # Pipeline: Act stream is sigma_g(t), tanh_{g-1}(t), sigma_{g+1}(t),
        # ... -- back(g) is emitted one group-slot after front(g) so the tanh
        # never blocks the next group's sigmoid on the in-order Act engine.
        # x-matmuls for a group's next step are emitted right after its back
        # phase (two slots ahead of the h-matmuls that accumulate onto them).
        pending = None  # group whose back phase is still to emit
        for t in range(t_steps):
            for g in range(G):
                emit_front(g, t)
                if pending is not None:
                    emit_back(pending)
                pending = g
        emit_back(pending)

        # ---------------- epilogue: out = h_flat @ W_eff + b_eff
        es.close()  # release psum + intermediate pools
        # copy group h tiles into one tile for stride-55 column gathers
        epi = top.enter_context(tc.tile_pool(name="epi", bufs=1))
        rall = epi.tile([128, NCT], GDT, name="rall")
        for g in range(G):
            nc.vector.tensor_copy(rall[:, NCOL * g:NCOL * (g + 1)], Rg[g][:])
        epips = top.enter_context(
            tc.tile_pool(name="epips", bufs=1, space="PSUM"))
        ps_o = epips.tile([FC_OUT, 512], F32, name="ps_o")

        # col(chain, px) = chain*55 + px; per px: 8 chains at stride 55
        rv = rall[:, 0:NCHAIN * P].rearrange("p (ch q) -> p ch q", q=P)
        for pi in range(P):
            nc.tensor.matmul(ps_o[:, 0:NCHAIN],
                             weff_sb[:, FC_OUT * pi:FC_OUT * (pi + 1)],
                             rv[:, :, pi],
                             start=(pi == 0), stop=(pi == P - 1))

        outs = epi.tile([FC_OUT, NCHAIN], F32, name="outs")
        nc.scalar.activation(outs[:], ps_o[:, 0:NCHAIN],
                             mybir.ActivationFunctionType.Identity,
                             bias=beff_sb[:])
        # outs cols are chain = input*BL + sample: straight copy out
        dst = bass.AP(out_d.tensor, 0,
                      [[1, FC_OUT], [BL * FC_OUT, 2], [FC_OUT, BL]])
        nc.sync.dma_start(out=dst, in_=outs.rearrange("p (i s) -> p i s", i=2))

    nc.compile()
    _BUILD_CACHE[key] = nc
    return nc


# ---------------------------------------------------------------- host prep
def _host_prep(inputs, t_steps):
    """Build per-core input maps from the full problem inputs."""
    f = lambda k: np.asarray(inputs[k], np.float32)
    x1, x2 = f("x1"), f("x2")
    wh = [f("wh1"), f("wh2")]
    wx = [f("wx1"), f("wx2")]
    bsum = [f("bx1") + f("bh1"), f("bx2") + f("bh2")]

    n_tb = (t_steps + 63) // 64

    # wh block-diag per gate: [128, 4*128]
    wh_host = np.zeros((128, 512), np.float32)
    for k, gate in enumerate(BANKS):
        a, b = GATE_SL[gate]
        m = 2.0 if gate == "g" else 1.0
        wh_host[0:64, 128 * k:128 * k + 64] = wh[0][:, a:b] * m
        wh_host[64:128, 128 * k + 64:128 * k + 128] = wh[1][:, a:b] * m

    # wx: one K=12 stationary per bank: rows (per 32-block) 0-5 = even gate
    # (cell1 x0, x1, bias; cell2 x0r, x1r, bias), rows 6-11 = odd gate
    wx_host = np.zeros((128, 256), np.float32)
    for k, gate in enumerate(BANKS):
        a, b = GATE_SL[gate]
        m = 2.0 if gate == "g" else 1.0
        blk = np.zeros((6, 128), np.float32)
        blk[0:2, 0:64] = wx[0][:, a:b] * m
        blk[2, 0:64] = bsum[0][a:b] * m
        blk[3:5, 64:128] = wx[1][:, a:b] * m
        blk[5, 64:128] = bsum[1][a:b] * m
        for j in range(n_tb):
            r0 = 32 * j + 6 * (k % 2)
            wx_host[r0:r0 + 6, 128 * (k // 2):128 * (k // 2) + 128] = blk

    # collapsed FC stack (f64 accumulation)
    Wf = (f("fw2").astype(np.float64) @ f("fw3").astype(np.float64)
          @ f("fw4").astype(np.float64) @ f("fw5").astype(np.float64))
    bf = (((f("fb2").astype(np.float64) @ f("fw3").astype(np.float64)
            + f("fb3").astype(np.float64)) @ f("fw4").astype(np.float64)
           + f("fb4").astype(np.float64)) @ f("fw5").astype(np.float64)
          + f("fb5").astype(np.float64))
    weff_host = Wf.astype(np.float32).reshape(2, 64, P, FC_OUT).reshape(
        128, P * FC_OUT)
    beff_host = bf.astype(np.float32).reshape(FC_OUT, 1)

    whd = wh_host.astype(_np(WDT))
    wxd = wx_host.astype(_np(WDT))
    weffd = weff_host.astype(_np(WDT))

    in_maps = []
    for core in range(N_CORES):
        s0 = BL * core
        xs1 = x1[s0:s0 + BL, :t_steps].reshape(BL, t_steps, IC, P)
        xs2 = x2[s0:s0 + BL, :t_steps].reshape(BL, t_steps, IC, P)
        # columns: chain-major, chain = input*BL + sample; col = chain*55+px
        xcols = np.concatenate([xs1, xs2], 0)          # (8, T, IC, P)
        xcols = xcols.transpose(2, 1, 0, 3).reshape(IC, t_steps, NCHAIN * P)
        xcols = np.concatenate(
            [xcols, np.zeros((IC, t_steps, 1), np.float32)], -1)  # pad col
        xrev = xcols[:, ::-1]                          # cell2: reversed time

        # block-diagonal: rows 0-5 carry the data for cols [0:NCOL] (even
        # gate), rows 6-11 the same data for cols [NCOL:2*NCOL] (odd gate)
        xg = np.zeros((n_tb, 12, G, 64, 2 * NCOL), np.float32)
        for j in range(n_tb):
            hi = min(64, t_steps - 64 * j)
            for g in range(G):
                sl = slice(NCOL * g, NCOL * (g + 1))
                for half in range(2):
                    cs = slice(NCOL * half, NCOL * (half + 1))
                    r = 6 * half
                    xg[j, r + 0:r + 2, g, :hi, cs] = \
                        xcols[:, 64 * j:64 * j + hi, sl]
                    xg[j, r + 2, g, :hi, cs] = 1.0
                    xg[j, r + 3:r + 5, g, :hi, cs] = \
                        xrev[:, 64 * j:64 * j + hi, sl]
                    xg[j, r + 5, g, :hi, cs] = 1.0
        in_maps.append({
            "xg": xg.astype(_np(XDT)),
            "wh": whd,
            "wx": wxd,
            "weff": weffd,
            "beff": beff_host,
        })
    return in_maps


# ---------------------------------------------------------------- entry point
def _run(inputs, t_steps=T_FULL):
    nc = _build(t_steps)
    in_maps = _host_prep(inputs, t_steps)
    res = run_bass_kernel_spmd(nc, in_maps, list(range(N_CORES)))
    out1 = np.concatenate([res.results[i]["out"][0] for i in range(N_CORES)], 0)
    out2 = np.concatenate([res.results[i]["out"][1] for i in range(N_CORES)], 0)
    return out1.astype(np.float32), out2.astype(np.float32)


def kernel(**inputs):
    return _run(inputs, T_FULL)


# revision 22
# speedup vs baseline: 1.0863x; 1.0863x over previous
"""Trainium2 Bass kernel for nn_ConvLSTMNet (bidirectional per-pixel ConvLSTM + FC stack).

Strategy (v2)
-------------
* Data-parallel over batch: 8 cores x 4 samples; each core runs both x1 and x2
  sub-forwards (shared weights) = 8 chains x 55 px = 440 recurrence columns
  (+1 zero pad col), all independent until the epilogue.
* FC stack 7040->3400->1000->500->50 has no nonlinearities -> collapsed on host
  into one 7040->50 matrix (f64 accumulate); tiny on-device GEMM at the end.
* Columns split into G=3 phase groups of 147 cols, pipelined so the Activation
  engine stream is sigma_g(t), tanh_{g-1}(t), sigma_{g+1}(t), ... with each
  instruction's input ready before the engine reaches it (Act is the
  throughput-limiting engine: cost = elems*0.83ns + ~185ns init per instr).
* Both cells merged per matmul via block-diagonal weights: per gate one
  K=128 h-matmul ([wh1 0; 0 wh2], rhs = [h1;h2]) and one K=6 x-matmul
  ([wx1|b1 0; 0 wx2|b2] with a ones-row carrying the bias; cell2's x data
  pre-reversed in time on the host) -> 8 matmuls per group-step instead of 16.
* PSUM: 2 gates per bank ([f|i] and [o|g]), 2 banks per group, 6 banks total.
  One sigmoid per group-step over all 4 gates (tanh(g) computed as
  2*sigmoid(2g)-1 with pre-doubled g weights), one Tanh(c), 4 DVE ops.
"""

import os
import sys

try:
    import concourse.bass  # noqa: F401  (provided by the environment boot)
except ImportError:  # fallback for bare environments
    sys.path.insert(0, "/opt/trn_rl_repo")

import numpy as np
import ml_dtypes

import concourse.bass as bass
import concourse.bacc as bacc
import concourse.tile as tile
from concourse import mybir
from concourse.bass_utils import run_bass_kernel_spmd

# ---------------------------------------------------------------- constants
B, T_FULL, IC, H, W = 32, 256, 2, 5, 11
P = H * W            # 55
HC = 64
N_CORES = 8
BL = B // N_CORES    # 4 samples per core
NCHAIN = 2 * BL      # 8 chains per core (x1/x2 x samples)
G = 3                # phase groups (column split)
NCOL = 147           # columns per group (3*147 = 441 = 440 + 1 pad)
NCT = G * NCOL       # 441
FC_OUT = 50

GATE_SL = {"i": (0, 64), "f": (64, 128), "o": (128, 192), "g": (192, 256)}
BANKS = ["f", "i", "o", "g"]  # per-group gate order; "g" is pre-doubled

F32 = mybir.dt.float32
BF16 = mybir.dt.bfloat16

GDT = BF16   # sigmoid/tanh outputs, h/R
CDT = BF16 if os.environ.get("K_CDT", "f32") == "bf16" else F32
XDT = BF16   # staged x data
WDT = BF16   # recurrence weights

_NPDT = {F32: np.float32, BF16: ml_dtypes.bfloat16}


def _np(dt):
    return _NPDT[dt]


# ---------------------------------------------------------------- device build
_BUILD_CACHE = {}


def _build(t_steps: int):
    """Build + compile the per-core Bass module (cached)."""
    key = (t_steps,)
    if key in _BUILD_CACHE:
        return _BUILD_CACHE[key]

    n_tb = (t_steps + 63) // 64  # number of 64-step time blocks (4 for T=256)
    assert n_tb <= 4

    nc = bacc.Bacc("TRN2", target_bir_lowering=False, debug=False,
                   num_devices=N_CORES)

    # xg: per time-block j (partition rows 32j..32j+11): a [12, 2*NCOL]
    # block-diagonal x layout per (group, step-in-block): rows 0-5 hold
    # (x0, x1, ones, x0r, x1r, ones) for the even gate's columns [0:NCOL],
    # rows 6-11 the same data for the odd gate's columns [NCOL:2*NCOL]
    # (zeros elsewhere). One K=12 x-matmul then fills a whole psum bank
    # (both gates), so each bank gets exactly ONE start_tensor_calc per
    # step (start marks the whole 2KB zero region pending-zero) and the
    # x-matmuls can dispatch before R is ready.
    xg_d = nc.dram_tensor("xg", [n_tb, 12, G, 64, 2 * NCOL], XDT,
                          kind="ExternalInput").ap()
    wh_d = nc.dram_tensor("wh", [128, 512], WDT, kind="ExternalInput").ap()
    wx_d = nc.dram_tensor("wx", [128, 256], WDT, kind="ExternalInput").ap()
    weff_d = nc.dram_tensor("weff", [128, P * FC_OUT], WDT,
                            kind="ExternalInput").ap()
    beff_d = nc.dram_tensor("beff", [FC_OUT, 1], F32, kind="ExternalInput").ap()
    out_d = nc.dram_tensor("out", [2, BL, FC_OUT], F32,
                           kind="ExternalOutput").ap()

    from contextlib import ExitStack

    with tile.TileContext(nc) as tc, ExitStack() as top:
        # ---------------- persistent SBUF tiles
        singles = top.enter_context(tc.tile_pool(name="singles", bufs=1))
        xg_sb = singles.tile([128, G, 64, 2 * NCOL], XDT, name="xg_sb")
        wh_sb = singles.tile([128, 512], WDT, name="wh_sb")
        wx_sb = singles.tile([128, 256], WDT, name="wx_sb")
        weff_sb = singles.tile([128, P * FC_OUT], WDT, name="weff_sb")
        beff_sb = singles.tile([FC_OUT, 1], F32, name="beff_sb")

        Rg, cg = [], []
        for g in range(G):
            Rg.append(singles.tile([128, NCOL], GDT, name=f"R{g}"))
            cg.append(singles.tile([128, NCOL], CDT, name=f"c{g}"))

        # ---------------- loads + state init
        # Small weights first (the first x-matmul needs wx), on separate
        # engine queues so transfers overlap; first 8 steps of xg split out
        # so compute starts early; weff (epilogue-only) last.
        nc.scalar.dma_start(out=wx_sb[:], in_=wx_d[:])
        nc.scalar.dma_start(out=wh_sb[:], in_=wh_d[:])
        nc.scalar.dma_start(out=beff_sb[:], in_=beff_d[:])
        nc.sync.dma_start(out=xg_sb[0:12, :, 0:8], in_=xg_d[0][:, :, 0:8])
        nc.sync.dma_start(out=xg_sb[0:12, :, 8:64], in_=xg_d[0][:, :, 8:64])
        for j in range(1, n_tb):
            nc.sync.dma_start(out=xg_sb[32 * j:32 * j + 12],
                              in_=xg_d[j])
        nc.gpsimd.dma_start(out=weff_sb[:], in_=weff_d[:])
        for g in range(G):
            nc.vector.memset(Rg[g][:], 0.0)
            nc.vector.memset(cg[g][:], 0.0)

        # ---------------- pools for psum + per-step intermediates
        es = ExitStack()
        pspool = es.enter_context(
            tc.tile_pool(name="psmain", bufs=1, space="PSUM"))
        # per group: 2 banks, 2 gates per bank: [f|i], [o|g]
        ps = [pspool.tile([128, 1024], F32, name=f"ps{g}") for g in range(G)]
        pools = {}
        for g in range(G):
            for nm in ("sg", "vp", "u", "tct"):
                pools[(nm, g)] = es.enter_context(
                    tc.tile_pool(name=f"{nm}{g}", bufs=2))

        sg_cur = [None] * G   # current sigmoid-output tile per group

        def bank_ap(g: int, k: int):
            return ps[g][:, 512 * (k // 2) + NCOL * (k % 2):
                         512 * (k // 2) + NCOL * (k % 2) + NCOL]

        def emit_front(g: int, t: int):
            """mm(g,t), sigma(g,t), DVE vp/u/c (g,t).

            The 2 bank-wide x-matmuls are emitted before the h-matmuls:
            they depend only on sigma(g,t-1) having read the banks, so the
            in-order PE SEQ dispatches them early and only the 4 h-matmuls
            sit between R(g,t-1) landing and sigma(g,t)."""
            j, s = t // 64, t % 64
            r0 = 32 * j
            for b in range(2):
                nc.tensor.matmul(ps[g][:, 512 * b:512 * b + 2 * NCOL],
                                 wx_sb[r0:r0 + 12, 128 * b:128 * b + 128],
                                 xg_sb[r0:r0 + 12, g, s, :],
                                 start=True, stop=False,
                                 tile_position=(r0, 0))
            for k in range(4):
                nc.tensor.matmul(bank_ap(g, k),
                                 wh_sb[:, 128 * k:128 * k + 128],
                                 Rg[g][:],
                                 start=False, stop=(k % 2 == 1))

            sg = pools[("sg", g)].tile([128, 4, NCOL], GDT, name=f"sgt{g}")
            psv = ps[g].rearrange("p (b n) -> p b n", b=2)[:, :, 0:2 * NCOL]
            nc.scalar.activation(sg.rearrange("p f n -> p (f n)"), psv,
                                 mybir.ActivationFunctionType.Sigmoid)
            sg_cur[g] = sg

            # gate views: bank0 = [f|i], bank1 = [o|g]
            s_f, s_i = sg[:, 0, :], sg[:, 1, :]
            s_g = sg[:, 3, :]
            vp = pools[("vp", g)].tile([128, NCOL], GDT, name=f"vpt{g}")
            # vp = (sig(2g) - 0.5) * sig(i)   ( = tanh(g)*sig(i)/2 )
            nc.vector.scalar_tensor_tensor(vp[:], s_g, 0.5, s_i,
                                           mybir.AluOpType.subtract,
                                           mybir.AluOpType.mult)
            u = pools[("u", g)].tile([128, NCOL], CDT, name=f"ut{g}")
            nc.vector.tensor_mul(u[:], s_f, cg[g][:])     # sig(f)*c
            # c = 2*vp + u
            nc.vector.scalar_tensor_tensor(cg[g][:], vp[:], 2.0, u[:],
                                           mybir.AluOpType.mult,
                                           mybir.AluOpType.add)

        def emit_back(g: int):
            """tanh(c_g) then h(g) = sig(o)*tanh(c) -> R(g).

            Emitted one group-slot after front(g): the Act engine reaches this
            tanh only after the next group's sigmoid, by which time c_g is
            ready -- no in-order blocking on the Act queue.
            """
            tct = pools[("tct", g)].tile([128, NCOL], GDT, name=f"tctt{g}")
            nc.scalar.activation(tct[:], cg[g][:],
                                 mybir.ActivationFunctionType.Tanh)
            s_o = sg_cur[g][:, 2, :]
            nc.vector.tensor_mul(Rg[g][:], s_o, tct[:])

        # Pipeline: Act stream is sigma_g(t), tanh_{g-1}(t), sigma_{g+1}(t),
        # ... -- back(g) is emitted one group-slot after front(g) so the tanh
        # never blocks the next group's sigmoid on the in-order Act engine.
        # x-matmuls for a group's next step are emitted right after its back
        # phase (two slots ahead of the h-matmuls that accumulate onto them).
        pending = None  # group whose back phase is still to emit
        for t in range(t_steps):
            for g in range(G):
                emit_front(g, t)
                if pending is not None:
                    emit_back(pending)
                pending = g
        emit_back(pending)

        # ---------------- epilogue: out = h_flat @ W_eff + b_eff
        es.close()  # release psum + intermediate pools
        # copy group h tiles into one tile for stride-55 column gathers
        epi = top.enter_context(tc.tile_pool(name="epi", bufs=1))
        rall = epi.tile([128, NCT], GDT, name="rall")
        for g in range(G):
            nc.vector.tensor_copy(rall[:, NCOL * g:NCOL * (g + 1)], Rg[g][:])
        epips = top.enter_context(
            tc.tile_pool(name="epips", bufs=1, space="PSUM"))
        ps_o = epips.tile([FC_OUT, 512], F32, name="ps_o")

        # col(chain, px) = chain*55 + px; per px: 8 chains at stride 55
        rv = rall[:, 0:NCHAIN * P].rearrange("p (ch q) -> p ch q", q=P)
        for pi in range(P):
            nc.tensor.matmul(ps_o[:, 0:NCHAIN],
                             weff_sb[:, FC_OUT * pi:FC_OUT * (pi + 1)],
                             rv[:, :, pi],
                             start=(pi == 0), stop=(pi == P - 1))

        outs = epi.tile([FC_OUT, NCHAIN], F32, name="outs")
        nc.scalar.activation(outs[:], ps_o[:, 0:NCHAIN],
                             mybir.ActivationFunctionType.Identity,
                             bias=beff_sb[:])
        # outs cols are chain = input*BL + sample: straight copy out
        dst = bass.AP(out_d.tensor, 0,
                      [[1, FC_OUT], [BL * FC_OUT, 2], [FC_OUT, BL]])
        nc.sync.dma_start(out=dst, in_=outs.rearrange("p (i s) -> p i s", i=2))

    nc.compile()
    _BUILD_CACHE[key] = nc
    return nc


# ---------------------------------------------------------------- host prep
def _host_prep(inputs, t_steps):
    """Build per-core input maps from the full problem inputs."""
    f = lambda k: np.asarray(inputs[k], np.float32)
    x1, x2 = f("x1"), f("x2")
    wh = [f("wh1"), f("wh2")]
    wx = [f("wx1"), f("wx2")]
    bsum = [f("bx1") + f("bh1"), f("bx2") + f("bh2")]

    n_tb = (t_steps + 63) // 64

    # wh block-diag per gate: [128, 4*128]
    wh_host = np.zeros((128, 512), np.float32)
    for k, gate in enumerate(BANKS):
        a, b = GATE_SL[gate]
        m = 2.0 if gate == "g" else 1.0
        wh_host[0:64, 128 * k:128 * k + 64] = wh[0][:, a:b] * m
        wh_host[64:128, 128 * k + 64:128 * k + 128] = wh[1][:, a:b] * m

    # wx: one K=12 stationary per bank: rows (per 32-block) 0-5 = even gate
    # (cell1 x0, x1, bias; cell2 x0r, x1r, bias), rows 6-11 = odd gate
    wx_host = np.zeros((128, 256), np.float32)
    for k, gate in enumerate(BANKS):
        a, b = GATE_SL[gate]
        m = 2.0 if gate == "g" else 1.0
        blk = np.zeros((6, 128), np.float32)
        blk[0:2, 0:64] = wx[0][:, a:b] * m
        blk[2, 0:64] = bsum[0][a:b] * m
        blk[3:5, 64:128] = wx[1][:, a:b] * m
        blk[5, 64:128] = bsum[1][a:b] * m
        for j in range(n_tb):
            r0 = 32 * j + 6 * (k % 2)
            wx_host[r0:r0 + 6, 128 * (k // 2):128 * (k // 2) + 128] = blk

    # collapsed FC stack (f64 accumulation)
    Wf = (f("fw2").astype(np.float64) @ f("fw3").astype(np.float64)
          @ f("fw4").astype(np.float64) @ f("fw5").astype(np.float64))
    bf = (((f("fb2").astype(np.float64) @ f("fw3").astype(np.float64)
            + f("fb3").astype(np.float64)) @ f("fw4").astype(np.float64)
           + f("fb4").astype(np.float64)) @ f("fw5").astype(np.float64)
          + f("fb5").astype(np.float64))
    weff_host = Wf.astype(np.float32).reshape(2, 64, P, FC_OUT).reshape(
        128, P * FC_OUT)
    beff_host = bf.astype(np.float32).reshape(FC_OUT, 1)

    whd = wh_host.astype(_np(WDT))
    wxd = wx_host.astype(_np(WDT))
    weffd = weff_host.astype(_np(WDT))

    in_maps = []
    for core in range(N_CORES):
        s0 = BL * core
        xs1 = x1[s0:s0 + BL, :t_steps].reshape(BL, t_steps, IC, P)
        xs2 = x2[s0:s0 + BL, :t_steps].reshape(BL, t_steps, IC, P)
        # columns: chain-major, chain = input*BL + sample; col = chain*55+px
        xcols = np.concatenate([xs1, xs2], 0)          # (8, T, IC, P)
        xcols = xcols.transpose(2, 1, 0, 3).reshape(IC, t_steps, NCHAIN * P)
        xcols = np.concatenate(
            [xcols, np.zeros((IC, t_steps, 1), np.float32)], -1)  # pad col
        xrev = xcols[:, ::-1]                          # cell2: reversed time

        # block-diagonal: rows 0-5 carry the data for cols [0:NCOL] (even
        # gate), rows 6-11 the same data for cols [NCOL:2*NCOL] (odd gate)
        xg = np.zeros((n_tb, 12, G, 64, 2 * NCOL), np.float32)
        for j in range(n_tb):
            hi = min(64, t_steps - 64 * j)
            for g in range(G):
                sl = slice(NCOL * g, NCOL * (g + 1))
                for half in range(2):
                    cs = slice(NCOL * half, NCOL * (half + 1))
                    r = 6 * half
                    xg[j, r + 0:r + 2, g, :hi, cs] = \
                        xcols[:, 64 * j:64 * j + hi, sl]
                    xg[j, r + 2, g, :hi, cs] = 1.0
                    xg[j, r + 3:r + 5, g, :hi, cs] = \
                        xrev[:, 64 * j:64 * j + hi, sl]
                    xg[j, r + 5, g, :hi, cs] = 1.0
        in_maps.append({
            "xg": xg.astype(_np(XDT)),
            "wh": whd,
            "wx": wxd,
            "weff": weffd,
            "beff": beff_host,
        })
    return in_maps


# ---------------------------------------------------------------- entry point
def _run(inputs, t_steps=T_FULL):
    nc = _build(t_steps)
    in_maps = _host_prep(inputs, t_steps)
    res = run_bass_kernel_spmd(nc, in_maps, list(range(N_CORES)))
    out1 = np.concatenate([res.results[i]["out"][0] for i in range(N_CORES)], 0)
    out2 = np.concatenate([res.results[i]["out"][1] for i in range(N_CORES)], 0)
    return out1.astype(np.float32), out2.astype(np.float32)


def kernel(**inputs):
    return _run(inputs, T_FULL)
